# revision 30
# baseline (speedup 1.0000x reference)
"""Segment-sum (scatter-add) kernel for Trainium2, 8 NeuronCores.

Strategy
--------
out[n, :] = sum_{e : index[e] == n} input[e, :]   (N=50000 segments, d=64)

Host side (data movement / re-encoding only):
  1. argsort(index) -> edges grouped by destination segment.
  2. Greedily pack *whole segments* (in id order) into fixed-capacity
     "chunks": each chunk covers <= 32 consecutive segment ids and
     <= 1024 edges (= 8 tiles x 128 edge rows).  Fill rate ~98%.
  3. Chunks are split contiguously across the 8 cores (each core owns a
     disjoint segment-id range -> no inter-core reduction needed).
  4. Edge rows are quantized fp32 -> fp8e3 (E3M4) with *error-diffusion*
     within each (segment, feature) stream: the rounding error of each
     edge is carried into the next edge of the same segment, so the
     quantized per-segment sums track the exact sums to ~one quantum
     (measured rel err 4.2e-3 vs the 2e-2 gate).  This quarters HBM
     traffic vs fp32 (the kernel is DMA-bound).
  5. Per core, edge rows are laid out partition-major so every DMA is a
     dense [128, W] strip.

Device side (all FLOPs):
  Chunks are processed in quads (4 chunks) using 4x column tiling of
  the 128x128 PE array (128x32 mode): chunk 4q+j's one-hot [128 edges,
  32 segs] is the stationary operand of col tile j, its x rows [128, 64]
  the moving operand, accumulating into PSUM partitions 32j..32j+32,
  free cols q*64..(q+1)*64.  8 quads fill one PSUM bank [128, 512].
  One-hots are built on the Vector engine for a whole strip in one
  batched is_equal.  Flush is a full-128-partition ScalarE copy
  PSUM->SBUF; output streamed per strip on the Scalar DMA ring.

Host finalization: place per-chunk row blocks into the [50000, 64]
output (pure scatter placement; np.add.at only if a segment ever had
to be split across chunks, which does not happen at these shapes).
"""

import os
import sys

for _p in ("/opt/trn_rl_repo", "/opt/pypackages"):
    if _p not in sys.path:
        sys.path.append(_p)

import numpy as np
import ml_dtypes

import concourse.mybir as mybir
from concourse import bacc
from concourse.mybir import AluOpType
from concourse.tile import TileContext
from concourse.bass_utils import run_bass_kernel_spmd

N_CORES = 8
P = 128               # partitions / contraction dim per tile
D = 64                # feature dim
SEGS_PER_CHUNK = 32   # one-hot width / psum partitions per col tile
TILES_PER_CHUNK = 8
EDGES_PER_CHUNK = TILES_PER_CHUNK * P   # 1024
CHUNKS_PER_QUAD = 4   # col-tiling factor
MAX_STRIP_CHUNKS = 32  # chunks per strip = 8 quads = two PSUM banks
PSUM_BANK_F32 = 512   # one PSUM bank per partition, in fp32 elements

F32 = mybir.dt.float32
F16 = mybir.dt.float16

QUANT = os.environ.get("QUANT", "f8")
if QUANT == "f8":
    XDT = mybir.dt.float8e3
    NP_XDT = ml_dtypes.float8_e3m4
else:
    XDT = mybir.dt.float16
    NP_XDT = np.float16

# one-hot dtype: f16 engages the DVE 16-bit fast path; matmul allows
# mixed fp16 stationary x fp8 moving
OH_DT = mybir.dt.float16 if os.environ.get("OHDT", "f16") == "f16" else XDT
# fraction of each strip's one-hot built on DVE (rest on GpSimd; walrus
# rejects is_equal on Pool, so default is DVE-only)
OH_DVE_FRAC = float(os.environ.get("OH_DVE_FRAC", "1.0"))


# --------------------------------------------------------------------------
# host-side packing
# --------------------------------------------------------------------------

def pack_chunks(index: np.ndarray, n_segments: int):
    """Group sorted edges into fixed-capacity chunks of whole segments.

    Returns (order, chunk_seg_base, chunk_nseg, chunk_edge_start, chunk_nedge).
    """
    index = np.asarray(index).astype(np.int64, copy=False).ravel()
    order = np.argsort(index, kind="stable")
    counts = np.bincount(index, minlength=n_segments)

    seg_base, nsegs, edge_start, nedges = [], [], [], []
    s = 0
    epos = 0
    counts_list = counts.tolist()
    while s < n_segments:
        c = counts_list[s]
        if c > EDGES_PER_CHUNK:
            # split one oversized segment across several chunks
            left = c
            while left > 0:
                take = min(left, EDGES_PER_CHUNK)
                seg_base.append(s); nsegs.append(1)
                edge_start.append(epos); nedges.append(take)
                epos += take
                left -= take
            s += 1
            continue
        base = s
        tot = 0
        ns = 0
        while (
            s < n_segments
            and ns < SEGS_PER_CHUNK
            and tot + counts_list[s] <= EDGES_PER_CHUNK
        ):
            tot += counts_list[s]
            ns += 1
            s += 1
        seg_base.append(base); nsegs.append(ns)
        edge_start.append(epos); nedges.append(tot)
        epos += tot
    return (
        order,
        np.array(seg_base, dtype=np.int64),
        np.array(nsegs, dtype=np.int64),
        np.array(edge_start, dtype=np.int64),
        np.array(nedges, dtype=np.int64),
    )


def quantize_diffused(x_sorted: np.ndarray, idx_sorted: np.ndarray, n_segments: int):
    """Quantize to NP_XDT, carrying per-(segment, feature) rounding error
    forward through the segment's edge stream so segment sums stay exact
    to ~one quantum."""
    n_edges = x_sorted.shape[0]
    counts = np.bincount(idx_sorted, minlength=n_segments)
    starts = np.zeros(n_segments, dtype=np.int64)
    np.cumsum(counts[:-1], out=starts[1:])
    pos = np.arange(n_edges, dtype=np.int64) - starts[idx_sorted]
    order_r = np.argsort(pos, kind="stable")
    pc = np.bincount(pos)
    b = np.zeros(len(pc) + 1, dtype=np.int64)
    np.cumsum(pc, out=b[1:])
    q = np.empty_like(x_sorted, dtype=NP_XDT)
    carry = np.zeros((n_segments, x_sorted.shape[1]), dtype=np.float32)
    for r in range(len(pc)):
        rows = order_r[b[r]:b[r + 1]]
        segs = idx_sorted[rows]
        v = x_sorted[rows] + carry[segs]
        qv = v.astype(NP_XDT)
        q[rows] = qv
        carry[segs] = v - qv.astype(np.float32)
    return q


def build_device_arrays(input_np, index_np, n_segments):
    """Returns (per_core, in_maps, assemble)."""
    input_np = np.asarray(input_np, dtype=np.float32).reshape(-1, D)
    index_np = np.asarray(index_np).astype(np.int64, copy=False).ravel()
    n_edges = input_np.shape[0]

    order, seg_base, nseg, e_start, ne = pack_chunks(index_np, n_segments)
    n_chunks = len(seg_base)
    # same chunk count on every core (SPMD), whole quad PAIRS (the device
    # interleaves two quads across psum banks)
    per_core = -(-n_chunks // N_CORES)
    per_core = -(-per_core // (2 * CHUNKS_PER_QUAD)) * (2 * CHUNKS_PER_QUAD)
    total_chunks = per_core * N_CORES

    # slot id for every edge (chunks are contiguous runs in sorted order)
    edge_chunk = np.repeat(np.arange(n_chunks), ne)
    within = np.arange(n_edges) - np.repeat(e_start, ne)
    slot = edge_chunk * EDGES_PER_CHUNK + within

    idx_sorted = index_np[order]
    local_row = (idx_sorted - seg_base[edge_chunk]).astype(np.float16)

    x_sorted = input_np[order]
    xq = quantize_diffused(x_sorted, idx_sorted, n_segments)

    total_slots = total_chunks * EDGES_PER_CHUNK
    X_all = np.zeros((total_slots, D), dtype=NP_XDT)
    X_all[slot] = xq
    L_all = np.zeros(total_slots, dtype=np.float16)
    L_all[slot] = local_row  # small ints, exact in fp16

    n_tiles_core = per_core * TILES_PER_CHUNK
    iota = np.broadcast_to(
        np.arange(SEGS_PER_CHUNK, dtype=np.float16)[None, :], (P, SEGS_PER_CHUNK)
    ).copy()

    in_maps = []
    for c in range(N_CORES):
        lo_s = c * per_core * EDGES_PER_CHUNK
        hi_s = lo_s + per_core * EDGES_PER_CHUNK
        xt = X_all[lo_s:hi_s].reshape(n_tiles_core, P, D)
        xc = xt.transpose(1, 0, 2).reshape(P, n_tiles_core * D)
        lc = (
            L_all[lo_s:hi_s]
            .reshape(n_tiles_core, P)
            .transpose(1, 0)
        )
        # duplicate each tile's local-row value so the one-hot build's
        # innermost AP axis is a 2-element step-1 run (DVE 2x perf mode
        # needs step_x=+-1 / num_elem_x>1 on every operand)
        l2 = np.repeat(lc, 2, axis=1)
        in_maps.append(
            {
                "x": np.ascontiguousarray(xc),
                "l": np.ascontiguousarray(l2),
                "iota": iota,
            }
        )

    def assemble(core_outs):
        # core_outs: list of [128, (per_core//4) * D] f32; partition 32j+s of
        # quad col-block q holds (chunk 4q+j, local seg s)
        nq = per_core // CHUNKS_PER_QUAD
        rows = np.concatenate(
            [
                o.reshape(CHUNKS_PER_QUAD, SEGS_PER_CHUNK, nq, D)
                .transpose(2, 0, 1, 3)
                .reshape(per_core * SEGS_PER_CHUNK, D)
                for o in core_outs
            ],
            axis=0,
        )
        row_seg = np.full(total_chunks * SEGS_PER_CHUNK, -1, dtype=np.int64)
        for i in range(n_chunks):
            row_seg[
                i * SEGS_PER_CHUNK : i * SEGS_PER_CHUNK + nseg[i]
            ] = np.arange(seg_base[i], seg_base[i] + nseg[i])
        valid = row_seg >= 0
        out = np.zeros((n_segments, D), dtype=np.float32)
        targets = row_seg[valid]
        vals = rows[valid]
        if len(np.unique(targets)) == len(targets):
            out[targets] = vals
        else:  # a segment was split across chunks
            np.add.at(out, targets, vals)
        return out

    return per_core, in_maps, assemble


# --------------------------------------------------------------------------
# device kernel
# --------------------------------------------------------------------------

def build_bass(n_chunks: int):
    nc = bacc.Bacc(
        "TRN2", target_bir_lowering=False, debug=False, num_devices=N_CORES
    )
    assert n_chunks % CHUNKS_PER_QUAD == 0
    n_tiles = n_chunks * TILES_PER_CHUNK
    n_quads = n_chunks // CHUNKS_PER_QUAD
    max_strip_tiles = MAX_STRIP_CHUNKS * TILES_PER_CHUNK

    X = nc.dram_tensor("x", [P, n_tiles * D], XDT, kind="ExternalInput")
    L = nc.dram_tensor("l", [P, n_tiles * 2], F16, kind="ExternalInput")
    IOTA = nc.dram_tensor("iota", [P, SEGS_PER_CHUNK], F16, kind="ExternalInput")
    OUT = nc.dram_tensor(
        "out", [P, n_quads * D], F32, kind="ExternalOutput"
    )

    # ramp-up strip sizes (chunk counts, multiples of 4) so compute starts
    # after a small first DMA; ramp the tail down so the post-DMA drain is
    # short
    strips = []
    c = 0
    ramp = tuple(int(v) for v in os.environ.get("RAMP", "8,8,16").split(",") if v)
    for take in ramp:
        if c + take <= n_chunks:
            strips.append((c, take))
            c += take
    sizes = []
    rem = n_chunks - c
    while rem > MAX_STRIP_CHUNKS:
        sizes.append(MAX_STRIP_CHUNKS)
        rem -= MAX_STRIP_CHUNKS
    while rem >= 8:
        sizes.append(8)
        rem -= 8
    if rem > 0:
        sizes.append(rem)
    for take in sizes:
        strips.append((c, take))
        c += take
    assert all(ncs % 8 == 0 for _, ncs in strips), strips

    n_strips = len(strips)

    with TileContext(nc) as tc:
        with (
            tc.tile_pool(name="const", bufs=1) as cpool,
            tc.tile_pool(name="xin", bufs=5) as xpool,
            tc.tile_pool(name="lin", bufs=4) as lpool,
            tc.tile_pool(name="oh", bufs=3) as ohpool,
            tc.tile_pool(name="acc", bufs=3, space="PSUM") as ppool,
            tc.tile_pool(name="outp", bufs=4) as opool,
        ):
            iota_t = cpool.tile([P, SEGS_PER_CHUNK], F16)
            nc.scalar.dma_start(out=iota_t[:], in_=IOTA[:, :])

            def issue_fetch(si):
                """x/l strip fetches; x alternates the two HW DMA queues."""
                c0, ncs = strips[si]
                t0 = c0 * TILES_PER_CHUNK
                st = ncs * TILES_PER_CHUNK
                xq, lq = (nc.sync, nc.scalar) if si % 2 == 0 else (nc.scalar, nc.sync)
                xs = xpool.tile([P, max_strip_tiles * D], XDT, tag="xs")
                xq.dma_start(out=xs[:, : st * D], in_=X[:, t0 * D : (t0 + st) * D])
                l_t = lpool.tile([P, max_strip_tiles * 2], F16, tag="ls")
                lq.dma_start(out=l_t[:, : st * 2], in_=L[:, 2 * t0 : 2 * (t0 + st)])
                return xs, l_t

            def build_oh(si, l_t):
                """Batched one-hot for the whole strip on DVE.  The seg axis
                is pair-split (g = g2*2 + gp) so every operand's innermost
                axis is a 2-elem step-1 run -> DVE 2x perf mode."""
                c0, ncs = strips[si]
                st = ncs * TILES_PER_CHUNK
                G2 = SEGS_PER_CHUNK // 2
                oh = ohpool.tile(
                    [P, max_strip_tiles * SEGS_PER_CHUNK], OH_DT, tag="oh"
                )
                lb = (
                    l_t[:, : 2 * st]
                    .rearrange("p (t gp) -> p t gp", t=st, gp=2)
                    .unsqueeze(2)
                    .broadcast_to([P, st, G2, 2])
                )
                ib = (
                    iota_t[:]
                    .rearrange("p (g2 gp) -> p g2 gp", g2=G2, gp=2)
                    .unsqueeze(1)
                    .broadcast_to([P, st, G2, 2])
                )
                nc.vector.tensor_tensor(
                    oh[:, : st * SEGS_PER_CHUNK].rearrange(
                        "p (t g2 gp) -> p t g2 gp", t=st, g2=G2, gp=2
                    ),
                    ib,
                    lb,
                    AluOpType.is_equal,
                )
                return oh

            def issue_out(si, ost):
                c0, ncs = strips[si]
                nq = ncs // CHUNKS_PER_QUAD
                q0 = c0 // CHUNKS_PER_QUAD
                oq = nc.scalar if si % 2 == 0 else nc.sync
                oq.dma_start(
                    out=OUT[:, q0 * D : (q0 + nq) * D], in_=ost[:, : nq * D]
                )

            # software pipeline: fetch/one-hot run one strip ahead of the
            # matmuls; flushes ride DVE *after* the lookahead one-hot so they
            # never head-of-line-block it; output writes are issued two
            # strips late so their flush-wait never stalls x prefetch.
            xs_c, l_c = issue_fetch(0)
            oh_c = build_oh(0, l_c)
            pend = []  # (si, ost) awaiting out-DMA issue
            for si, (c0, ncs) in enumerate(strips):
                st = ncs * TILES_PER_CHUNK
                nq = ncs // CHUNKS_PER_QUAD
                xs, oh = xs_c, oh_c
                if si + 1 < n_strips:
                    xs_c, l_c = issue_fetch(si + 1)
                    oh_c = build_oh(si + 1, l_c)
                # two PSUM banks per strip: even quads in bank A, odd quads
                # in bank B; partition 32j+s, col (q//2)*64+d
                ps = ppool.tile([P, 2 * PSUM_BANK_F32], F32, tag="ps")
                # Interleave quad PAIRS across the two bank-halves of the
                # psum tile: a start=True matmul clears has_written for the
                # whole (partition-range x bank), so concurrent accumulation
                # chains must differ in partitions (the j col tiles) or in
                # bank (the pair halves).  8 matmuls sit between consecutive
                # uses of the same col tile.
                for qg in range(0, nq, 2):
                    pair = [qq for qq in (qg, qg + 1) if qq < nq]
                    for t in range(TILES_PER_CHUNK):
                        for qq in pair:
                            for j in range(CHUNKS_PER_QUAD):
                                ti = (
                                    qq * CHUNKS_PER_QUAD + j
                                ) * TILES_PER_CHUNK + t
                                # quad -> column: even quads in bank A,
                                # odd quads in bank B
                                pc = (qq % 2) * PSUM_BANK_F32 + (qq // 2) * D
                                nc.tensor.matmul(
                                    ps[
                                        32 * j : 32 * (j + 1),
                                        pc : pc + D,
                                    ],
                                    lhsT=oh[
                                        :,
                                        ti * SEGS_PER_CHUNK : (ti + 1)
                                        * SEGS_PER_CHUNK,
                                    ],
                                    rhs=xs[:, ti * D : (ti + 1) * D],
                                    start=(t == 0),
                                    stop=(t == TILES_PER_CHUNK - 1),
                                    tile_position=(0, 32 * j),
                                )
                ost = opool.tile(
                    [P, (MAX_STRIP_CHUNKS // CHUNKS_PER_QUAD) * D], F32, tag="ost"
                )
                # un-permute the bank-interleaved quads back to logical order
                ost2 = ost[:, : nq * D].rearrange(
                    "p (q2 r d) -> p q2 r d", q2=nq // 2, r=2, d=D
                )
                ps2 = ps[:].rearrange(
                    "p (r q2 d) -> p r q2 d", r=2, q2=PSUM_BANK_F32 // D, d=D
                )
                for r in range(2):
                    nc.vector.tensor_copy(
                        ost2[:, :, r, :], ps2[:, r, : nq // 2, :]
                    )
                pend.append((si, ost))
                if len(pend) > 2:
                    issue_out(*pend.pop(0))
            for item in pend:
                issue_out(*item)
    nc.compile()
    return nc


# --------------------------------------------------------------------------
# entry point
# --------------------------------------------------------------------------

def _run(input_np, index_np, n_segments, trace=False, trace_kwargs=None):
    per_core, in_maps, assemble = build_device_arrays(
        input_np, index_np, n_segments
    )
    nc = build_bass(per_core)
    res = run_bass_kernel_spmd(
        nc,
        in_maps,
        core_ids=list(range(N_CORES)),
        trace=trace,
        **(trace_kwargs or {}),
    )
    outs = [np.asarray(r["out"], dtype=np.float32) for r in res.results]
    return assemble(outs), res


def kernel(input, index):
    out, _ = _run(np.asarray(input), np.asarray(index), 50000)
    return out


# revision 32
# speedup vs baseline: 1.1337x; 1.1337x over previous
"""Segment-sum (scatter-add) kernel for Trainium2, 8 NeuronCores.

Strategy
--------
out[n, :] = sum_{e : index[e] == n} input[e, :]   (N=50000 segments, d=64)

Host side (data movement / re-encoding only):
  1. argsort(index) -> edges grouped by destination segment.
  2. Greedily pack *whole segments* (in id order) into fixed-capacity
     "chunks": each chunk covers <= 32 consecutive segment ids and
     <= 1024 edges (= 8 tiles x 128 edge rows).  Fill rate ~98%.
  3. Chunks are split contiguously across the 8 cores (each core owns a
     disjoint segment-id range -> no inter-core reduction needed).
  4. Edge rows are quantized fp32 -> fp8e3 (E3M4) with *error-diffusion*
     within each (segment, feature) stream: the rounding error of each
     edge is carried into the next edge of the same segment, so the
     quantized per-segment sums track the exact sums to ~one quantum
     (measured rel err 4.2e-3 vs the 2e-2 gate).  This quarters HBM
     traffic vs fp32 (the kernel is DMA-bound).
  5. Per core, edge rows are laid out partition-major so every DMA is a
     dense [128, W] strip.

Device side (all FLOPs):
  Chunks are processed in quads (4 chunks) using 4x column tiling of
  the 128x128 PE array (128x32 mode): chunk 4q+j's one-hot [128 edges,
  32 segs] is the stationary operand of col tile j, its x rows [128, 64]
  the moving operand, accumulating into PSUM partitions 32j..32j+32,
  free cols q*64..(q+1)*64.  8 quads fill one PSUM bank [128, 512].
  One-hots are built on the Vector engine for a whole strip in one
  batched is_equal.  Flush is a full-128-partition ScalarE copy
  PSUM->SBUF; output streamed per strip on the Scalar DMA ring.

Host finalization: place per-chunk row blocks into the [50000, 64]
output (pure scatter placement; np.add.at only if a segment ever had
to be split across chunks, which does not happen at these shapes).
"""

import os
import sys

for _p in ("/opt/trn_rl_repo", "/opt/pypackages"):
    if _p not in sys.path:
        sys.path.append(_p)

import numpy as np
import ml_dtypes

import concourse.mybir as mybir
from concourse import bacc
from concourse.mybir import AluOpType
from concourse.tile import TileContext
from concourse.bass_utils import run_bass_kernel_spmd

N_CORES = 8
P = 128               # partitions / contraction dim per tile
D = 64                # feature dim
SEGS_PER_CHUNK = 32   # one-hot width / psum partitions per col tile
TILES_PER_CHUNK = 8
EDGES_PER_CHUNK = TILES_PER_CHUNK * P   # 1024
CHUNKS_PER_QUAD = 4   # col-tiling factor
MAX_STRIP_CHUNKS = 16  # chunks per strip = 4 quads = half a PSUM bank
PSUM_BANK_F32 = 512   # one PSUM bank per partition, in fp32 elements

F32 = mybir.dt.float32
F16 = mybir.dt.float16

QUANT = os.environ.get("QUANT", "f8")
if QUANT == "f8":
    XDT = mybir.dt.float8e3
    NP_XDT = ml_dtypes.float8_e3m4
else:
    XDT = mybir.dt.float16
    NP_XDT = np.float16

# one-hot dtype: f16 engages the DVE 16-bit fast path; matmul allows
# mixed fp16 stationary x fp8 moving
OH_DT = mybir.dt.float16 if os.environ.get("OHDT", "f16") == "f16" else XDT
# fraction of each strip's one-hot built on DVE (rest on GpSimd; walrus
# rejects is_equal on Pool, so default is DVE-only)
OH_DVE_FRAC = float(os.environ.get("OH_DVE_FRAC", "1.0"))


# --------------------------------------------------------------------------
# host-side packing
# --------------------------------------------------------------------------

def pack_chunks(index: np.ndarray, n_segments: int):
    """Group sorted edges into fixed-capacity chunks of whole segments.

    Returns (order, chunk_seg_base, chunk_nseg, chunk_edge_start, chunk_nedge).
    """
    index = np.asarray(index).astype(np.int64, copy=False).ravel()
    order = np.argsort(index, kind="stable")
    counts = np.bincount(index, minlength=n_segments)

    seg_base, nsegs, edge_start, nedges = [], [], [], []
    s = 0
    epos = 0
    counts_list = counts.tolist()
    while s < n_segments:
        c = counts_list[s]
        if c > EDGES_PER_CHUNK:
            # split one oversized segment across several chunks
            left = c
            while left > 0:
                take = min(left, EDGES_PER_CHUNK)
                seg_base.append(s); nsegs.append(1)
                edge_start.append(epos); nedges.append(take)
                epos += take
                left -= take
            s += 1
            continue
        base = s
        tot = 0
        ns = 0
        while (
            s < n_segments
            and ns < SEGS_PER_CHUNK
            and tot + counts_list[s] <= EDGES_PER_CHUNK
        ):
            tot += counts_list[s]
            ns += 1
            s += 1
        seg_base.append(base); nsegs.append(ns)
        edge_start.append(epos); nedges.append(tot)
        epos += tot
    return (
        order,
        np.array(seg_base, dtype=np.int64),
        np.array(nsegs, dtype=np.int64),
        np.array(edge_start, dtype=np.int64),
        np.array(nedges, dtype=np.int64),
    )


def quantize_diffused(x_sorted: np.ndarray, idx_sorted: np.ndarray, n_segments: int):
    """Quantize to NP_XDT, carrying per-(segment, feature) rounding error
    forward through the segment's edge stream so segment sums stay exact
    to ~one quantum."""
    n_edges = x_sorted.shape[0]
    counts = np.bincount(idx_sorted, minlength=n_segments)
    starts = np.zeros(n_segments, dtype=np.int64)
    np.cumsum(counts[:-1], out=starts[1:])
    pos = np.arange(n_edges, dtype=np.int64) - starts[idx_sorted]
    order_r = np.argsort(pos, kind="stable")
    pc = np.bincount(pos)
    b = np.zeros(len(pc) + 1, dtype=np.int64)
    np.cumsum(pc, out=b[1:])
    q = np.empty_like(x_sorted, dtype=NP_XDT)
    carry = np.zeros((n_segments, x_sorted.shape[1]), dtype=np.float32)
    for r in range(len(pc)):
        rows = order_r[b[r]:b[r + 1]]
        segs = idx_sorted[rows]
        v = x_sorted[rows] + carry[segs]
        qv = v.astype(NP_XDT)
        q[rows] = qv
        carry[segs] = v - qv.astype(np.float32)
    return q


def build_device_arrays(input_np, index_np, n_segments):
    """Returns (per_core, in_maps, assemble)."""
    input_np = np.asarray(input_np, dtype=np.float32).reshape(-1, D)
    index_np = np.asarray(index_np).astype(np.int64, copy=False).ravel()
    n_edges = input_np.shape[0]

    order, seg_base, nseg, e_start, ne = pack_chunks(index_np, n_segments)
    n_chunks = len(seg_base)
    # same chunk count on every core (SPMD), whole quad PAIRS (the device
    # interleaves two quads across psum banks)
    per_core = -(-n_chunks // N_CORES)
    per_core = -(-per_core // (2 * CHUNKS_PER_QUAD)) * (2 * CHUNKS_PER_QUAD)
    total_chunks = per_core * N_CORES

    # slot id for every edge (chunks are contiguous runs in sorted order)
    edge_chunk = np.repeat(np.arange(n_chunks), ne)
    within = np.arange(n_edges) - np.repeat(e_start, ne)
    slot = edge_chunk * EDGES_PER_CHUNK + within

    idx_sorted = index_np[order]
    local_row = (idx_sorted - seg_base[edge_chunk]).astype(np.float16)

    x_sorted = input_np[order]
    xq = quantize_diffused(x_sorted, idx_sorted, n_segments)

    total_slots = total_chunks * EDGES_PER_CHUNK
    X_all = np.zeros((total_slots, D), dtype=NP_XDT)
    X_all[slot] = xq
    L_all = np.zeros(total_slots, dtype=np.float16)
    L_all[slot] = local_row  # small ints, exact in fp16

    n_tiles_core = per_core * TILES_PER_CHUNK
    iota = np.broadcast_to(
        np.arange(SEGS_PER_CHUNK, dtype=np.float16)[None, :], (P, SEGS_PER_CHUNK)
    ).copy()

    in_maps = []
    for c in range(N_CORES):
        lo_s = c * per_core * EDGES_PER_CHUNK
        hi_s = lo_s + per_core * EDGES_PER_CHUNK
        xt = X_all[lo_s:hi_s].reshape(n_tiles_core, P, D)
        xc = xt.transpose(1, 0, 2).reshape(P, n_tiles_core * D)
        lc = (
            L_all[lo_s:hi_s]
            .reshape(n_tiles_core, P)
            .transpose(1, 0)
        )
        # duplicate each tile's local-row value so the one-hot build's
        # innermost AP axis is a 2-element step-1 run (DVE 2x perf mode
        # needs step_x=+-1 / num_elem_x>1 on every operand)
        l2 = np.repeat(lc, 2, axis=1)
        in_maps.append(
            {
                "x": np.ascontiguousarray(xc),
                "l": np.ascontiguousarray(l2),
                "iota": iota,
            }
        )

    def assemble(core_outs):
        # core_outs: list of [128, (per_core//4) * D] f32; partition 32j+s of
        # quad col-block q holds (chunk 4q+j, local seg s)
        nq = per_core // CHUNKS_PER_QUAD
        rows = np.concatenate(
            [
                o.reshape(CHUNKS_PER_QUAD, SEGS_PER_CHUNK, nq, D)
                .transpose(2, 0, 1, 3)
                .reshape(per_core * SEGS_PER_CHUNK, D)
                for o in core_outs
            ],
            axis=0,
        )
        row_seg = np.full(total_chunks * SEGS_PER_CHUNK, -1, dtype=np.int64)
        for i in range(n_chunks):
            row_seg[
                i * SEGS_PER_CHUNK : i * SEGS_PER_CHUNK + nseg[i]
            ] = np.arange(seg_base[i], seg_base[i] + nseg[i])
        valid = row_seg >= 0
        out = np.zeros((n_segments, D), dtype=np.float32)
        targets = row_seg[valid]
        vals = rows[valid]
        if len(np.unique(targets)) == len(targets):
            out[targets] = vals
        else:  # a segment was split across chunks
            np.add.at(out, targets, vals)
        return out

    return per_core, in_maps, assemble


# --------------------------------------------------------------------------
# device kernel
# --------------------------------------------------------------------------

def build_bass(n_chunks: int):
    nc = bacc.Bacc(
        "TRN2", target_bir_lowering=False, debug=False, num_devices=N_CORES
    )
    assert n_chunks % CHUNKS_PER_QUAD == 0
    n_tiles = n_chunks * TILES_PER_CHUNK
    n_quads = n_chunks // CHUNKS_PER_QUAD
    max_strip_tiles = MAX_STRIP_CHUNKS * TILES_PER_CHUNK

    X = nc.dram_tensor("x", [P, n_tiles * D], XDT, kind="ExternalInput")
    L = nc.dram_tensor("l", [P, n_tiles * 2], F16, kind="ExternalInput")
    IOTA = nc.dram_tensor("iota", [P, SEGS_PER_CHUNK], F16, kind="ExternalInput")
    OUT = nc.dram_tensor(
        "out", [P, n_quads * D], F32, kind="ExternalOutput"
    )

    # ramp-up strip sizes (chunk counts, multiples of 4) so compute starts
    # after a small first DMA; ramp the tail down so the post-DMA drain is
    # short
    strips = []
    c = 0
    ramp = tuple(int(v) for v in os.environ.get("RAMP", "8,8").split(",") if v)
    for take in ramp:
        if c + take <= n_chunks:
            strips.append((c, take))
            c += take
    sizes = []
    rem = n_chunks - c
    while rem > MAX_STRIP_CHUNKS:
        sizes.append(MAX_STRIP_CHUNKS)
        rem -= MAX_STRIP_CHUNKS
    while rem >= 8:
        sizes.append(8)
        rem -= 8
    if rem > 0:
        sizes.append(rem)
    for take in sizes:
        strips.append((c, take))
        c += take
    assert all(ncs % 8 == 0 for _, ncs in strips), strips

    n_strips = len(strips)

    with TileContext(nc) as tc:
        with (
            tc.tile_pool(name="const", bufs=1) as cpool,
            tc.tile_pool(name="xin", bufs=5) as xpool,
            tc.tile_pool(name="lin", bufs=4) as lpool,
            tc.tile_pool(name="oh", bufs=3) as ohpool,
            tc.tile_pool(name="acc", bufs=3, space="PSUM") as ppool,
            tc.tile_pool(name="outp", bufs=4) as opool,
        ):
            iota_t = cpool.tile([P, SEGS_PER_CHUNK], F16)
            nc.scalar.dma_start(out=iota_t[:], in_=IOTA[:, :])

            def issue_fetch(si):
                """x/l strip fetches; x alternates the two HW DMA queues."""
                c0, ncs = strips[si]
                t0 = c0 * TILES_PER_CHUNK
                st = ncs * TILES_PER_CHUNK
                xq, lq = (nc.sync, nc.scalar) if si % 2 == 0 else (nc.scalar, nc.sync)
                xs = xpool.tile([P, max_strip_tiles * D], XDT, tag="xs")
                xq.dma_start(out=xs[:, : st * D], in_=X[:, t0 * D : (t0 + st) * D])
                l_t = lpool.tile([P, max_strip_tiles * 2], F16, tag="ls")
                lq.dma_start(out=l_t[:, : st * 2], in_=L[:, 2 * t0 : 2 * (t0 + st)])
                return xs, l_t

            def build_oh(si, l_t):
                """Batched one-hot for the whole strip on DVE.  The seg axis
                is pair-split (g = g2*2 + gp) so every operand's innermost
                axis is a 2-elem step-1 run -> DVE 2x perf mode."""
                c0, ncs = strips[si]
                st = ncs * TILES_PER_CHUNK
                G2 = SEGS_PER_CHUNK // 2
                oh = ohpool.tile(
                    [P, max_strip_tiles * SEGS_PER_CHUNK], OH_DT, tag="oh"
                )
                lb = (
                    l_t[:, : 2 * st]
                    .rearrange("p (t gp) -> p t gp", t=st, gp=2)
                    .unsqueeze(2)
                    .broadcast_to([P, st, G2, 2])
                )
                ib = (
                    iota_t[:]
                    .rearrange("p (g2 gp) -> p g2 gp", g2=G2, gp=2)
                    .unsqueeze(1)
                    .broadcast_to([P, st, G2, 2])
                )
                nc.vector.tensor_tensor(
                    oh[:, : st * SEGS_PER_CHUNK].rearrange(
                        "p (t g2 gp) -> p t g2 gp", t=st, g2=G2, gp=2
                    ),
                    ib,
                    lb,
                    AluOpType.is_equal,
                )
                return oh

            def issue_out(si, ost):
                c0, ncs = strips[si]
                nq = ncs // CHUNKS_PER_QUAD
                q0 = c0 // CHUNKS_PER_QUAD
                oq = nc.scalar if si % 2 == 0 else nc.sync
                oq.dma_start(
                    out=OUT[:, q0 * D : (q0 + nq) * D], in_=ost[:, : nq * D]
                )

            # software pipeline: fetch/one-hot run one strip ahead of the
            # matmuls; flushes ride DVE *after* the lookahead one-hot so they
            # never head-of-line-block it; output writes are issued two
            # strips late so their flush-wait never stalls x prefetch.
            xs_c, l_c = issue_fetch(0)
            oh_c = build_oh(0, l_c)
            pend = []  # (si, ost) awaiting out-DMA issue
            for si, (c0, ncs) in enumerate(strips):
                st = ncs * TILES_PER_CHUNK
                nq = ncs // CHUNKS_PER_QUAD
                xs, oh = xs_c, oh_c
                if si + 1 < n_strips:
                    xs_c, l_c = issue_fetch(si + 1)
                    oh_c = build_oh(si + 1, l_c)
                # one PSUM bank per strip: partition 32j+s, col q*64+d
                ps = ppool.tile([P, PSUM_BANK_F32], F32, tag="ps")
                # j-inner interleave only: a start=True matmul clears
                # has_written for the whole (partition-range x bank), so
                # concurrent accumulation chains must write disjoint
                # partitions -- the 4 col tiles do; quads stay sequential
                for qq in range(nq):
                    for t in range(TILES_PER_CHUNK):
                        for j in range(CHUNKS_PER_QUAD):
                            ti = (qq * CHUNKS_PER_QUAD + j) * TILES_PER_CHUNK + t
                            nc.tensor.matmul(
                                ps[
                                    32 * j : 32 * (j + 1),
                                    qq * D : (qq + 1) * D,
                                ],
                                lhsT=oh[
                                    :, ti * SEGS_PER_CHUNK : (ti + 1) * SEGS_PER_CHUNK
                                ],
                                rhs=xs[:, ti * D : (ti + 1) * D],
                                start=(t == 0),
                                stop=(t == TILES_PER_CHUNK - 1),
                                tile_position=(0, 32 * j),
                            )
                ost = opool.tile(
                    [P, (MAX_STRIP_CHUNKS // CHUNKS_PER_QUAD) * D], F32, tag="ost"
                )
                nc.vector.tensor_copy(ost[:, : nq * D], ps[:, : nq * D])
                pend.append((si, ost))
                if len(pend) > 2:
                    issue_out(*pend.pop(0))
            for item in pend:
                issue_out(*item)
    nc.compile()
    return nc


# --------------------------------------------------------------------------
# entry point
# --------------------------------------------------------------------------

def _run(input_np, index_np, n_segments, trace=False, trace_kwargs=None):
    per_core, in_maps, assemble = build_device_arrays(
        input_np, index_np, n_segments
    )
    nc = build_bass(per_core)
    res = run_bass_kernel_spmd(
        nc,
        in_maps,
        core_ids=list(range(N_CORES)),
        trace=trace,
        **(trace_kwargs or {}),
    )
    outs = [np.asarray(r["out"], dtype=np.float32) for r in res.results]
    return assemble(outs), res


def kernel(input, index):
    out, _ = _run(np.asarray(input), np.asarray(index), 50000)
    return out
